# revision 17
# baseline (speedup 1.0000x reference)
"""Trainium2 Bass kernel for the ALBEF-style GAT fusion model.

Shards batch B=64 across 8 NeuronCores (data parallel, 8 samples/core).
Key algorithmic facts exploited:
  - Only graph_embedding[:, 0, :] feeds the final head, so the fused GAT
    needs only attention column 0 (one softmax column), not the full
    639x639 attention.
  - GAT attention logits el/er are linear in the projection, so they are
    folded into the projection matmul as two extra weight columns.
  - The aggregation (alpha^T h) is emitted transposed (feat^T layout) so
    the next GAT's matmul needs no on-chip transpose.
Compute dtype: bf16 on the TensorEngine for the heavy matmuls, f32 for the
gumbel-gate / classifier head (argmax selection is precision-sensitive).
The per-sample work is software-pipelined in three stages so the
TensorEngine never stalls on the ScalarEngine softmax chain.
"""

import sys
import types

import numpy as np


def _install_ntff_hook():
    """Register the NTFF profiling hook that the trimmed antenv lacks.

    Without this, run_bass_kernel_spmd(trace=True) (or BASS_TRACE=1) dies on
    `import antenv.axon_hooks`. With it, tracing works via ctypes into
    libaxon_pjrt.so.
    """
    try:
        if 'antenv.axon_hooks' not in sys.modules:
            hooks = types.ModuleType('antenv.axon_hooks')
            _h = [None]
            hooks.set_axon_ntff_profile_hook = lambda h: _h.__setitem__(0, h)
            hooks.get_axon_ntff_profile_hook = lambda: _h[0]
            sys.modules['antenv.axon_hooks'] = hooks
            import antenv
            antenv.axon_hooks = hooks
        import trn_agent_boot.trn_boot as tb
        from antenv.axon_hooks import set_axon_ntff_profile_hook
        hook = tb._ntff_profile_via_ctypes('/opt/axon/libaxon_pjrt.so')
        if hook is not None:
            set_axon_ntff_profile_hook(hook)
    except Exception:
        pass


_install_ntff_hook()

import concourse.bacc as bacc
import concourse.tile as tile
from concourse import mybir
from concourse.bass_utils import run_bass_kernel_spmd

import ml_dtypes

F32 = mybir.dt.float32
BF16 = mybir.dt.bfloat16
U8 = mybir.dt.uint8
ALU = mybir.AluOpType
ACT = mybir.ActivationFunctionType

N_CORES = 8
B = 64
S = B // N_CORES          # samples per core
D = 768                   # hidden
FG = 256                  # GATConv out
DC = 128                  # classifier out
NI = 576                  # image nodes (CLS dropped)
NT1 = 64                  # text tokens incl CLS
NTN = 63                  # text nodes
NF = NI + NTN             # fused nodes = 639
KD = D // 128             # 6 k-tiles over hidden

IMG_ROWS = [128, 128, 128, 128, 64]          # image node tiles
FUS_ROWS = [128, 128, 128, 128, 127]         # fused node tiles
CG770 = [(0, 512), (512, 770)]
CG576 = [(0, 512), (512, 576)]
CG768 = [(0, 512), (512, 768)]

_COMPILED = None  # cached (nc, const_inputs)


def _host_consts():
    ident = np.eye(128, dtype=np.float32)
    # image diag mask, per i-tile: [5, 128, 576]
    mimg = np.ones((5, 128, NI), dtype=np.float32)
    for t in range(5):
        for p in range(128):
            i = t * 128 + p
            if i < NI:
                mimg[t, p, i] = 0.0
            else:
                mimg[t, p, :] = 0.0
    # text mask [128, 64]: token i=p%64 vs dst token j; zero CLS-as-src and diag
    mtxt = np.ones((128, NT1), dtype=np.float32)
    for p in range(128):
        i = p % 64
        mtxt[p, i] = 0.0
        if i == 0:
            mtxt[p, :] = 0.0
    return {
        "ident": ident,
        "mask_img": mimg.astype(ml_dtypes.bfloat16),
        "mask_txt": mtxt.astype(ml_dtypes.bfloat16),
    }


def _build():
    nc = bacc.Bacc()
    P = {}
    def dparam(name, shape, dtype=F32, out=False):
        P[name] = nc.declare_dram_parameter(name, list(shape), dtype, isOutput=out)
        return P[name]

    img = dparam("image_embeds", [S, 577, D])
    txt = dparam("text_hidden", [S, NT1, D])
    gum = dparam("gumbel", [S, 2, D])
    W_p = dparam("W_p", [D, D]); b_p = dparam("b_p", [D])
    al_p = dparam("al_p", [D]); ar_p = dparam("ar_p", [D])
    W_g = dparam("W_g", [D, FG]); b_g = dparam("b_g", [FG])
    al_g = dparam("al_g", [FG]); ar_g = dparam("ar_g", [FG])
    W_c = dparam("W_c", [FG, DC]); b_c = dparam("b_c", [DC])
    W_hp = dparam("W_hp", [DC, D]); b_hp = dparam("b_hp", [D])
    W_hm = dparam("W_hm", [2 * D, D]); b_hm = dparam("b_hm", [D])
    W_hg = dparam("W_hg", [2 * D, D]); b_hg = dparam("b_hg", [D])
    W_c1 = dparam("W_c1", [D, D]); b_c1 = dparam("b_c1", [D])
    W_c2 = dparam("W_c2", [D, 2]); b_c2 = dparam("b_c2", [2])
    ident = dparam("ident", [128, 128])
    mask_img = dparam("mask_img", [5, 128, NI], BF16)
    mask_txt = dparam("mask_txt", [128, NT1], BF16)
    out_p = dparam("out", [S, 2], out=True)

    with tile.TileContext(nc) as tc:
        with (
            tc.tile_pool(name="dram", bufs=1, space="DRAM") as dpool,
            tc.tile_pool(name="dstg", bufs=3, space="DRAM") as dstg,
            tc.tile_pool(name="consts", bufs=1) as cp,
            tc.tile_pool(name="work", bufs=2) as wp,
            tc.tile_pool(name="head", bufs=1) as hp,
            tc.tile_pool(name="wstream", bufs=6) as wsp,
            tc.tile_pool(name="wsf32", bufs=6) as wsf,
            tc.tile_pool(name="pmm", bufs=4, space="PSUM") as pmm,
        ):
            # ---------------- setup: constants ----------------
            ident_sb = cp.tile([128, 128], F32, tag="ident")
            nc.sync.dma_start(out=ident_sb[:], in_=ident[:])
            mimg_sb = cp.tile([128, 5, NI], BF16, tag="mimg")
            nc.sync.dma_start(out=mimg_sb[:], in_=mask_img.rearrange("t p j -> p t j"))
            mtxt_sb = cp.tile([128, NT1], BF16, tag="mtxt")
            nc.sync.dma_start(out=mtxt_sb[:], in_=mask_txt[:])

            onesf1 = cp.tile([1, 1], F32, tag="onesf1")
            nc.vector.memset(onesf1[:], 1.0)

            def bias_tile(par, n, tag, parts=128):
                # contiguous row load + K=1 matmul transpose (avoids 4B-strided DMA)
                brow2 = cp.tile([1, D], F32, tag="brow")
                nc.sync.dma_start(out=brow2[0:1, 0:n * parts], in_=par[:].unsqueeze(0))
                ps_b = pmm.tile([128, 6], F32, tag="mm")
                for m in range(n):
                    nc.tensor.matmul(ps_b[0:parts, m:m + 1], brow2[0:1, m * parts:(m + 1) * parts],
                                     onesf1[:], start=True, stop=True)
                t = cp.tile([parts, n], F32, tag=tag)
                nc.scalar.copy(out=t[:], in_=ps_b[0:parts, 0:n])
                return t
            bp_sb = bias_tile(b_p, 6, "bp")
            bpm1_sb = cp.tile([128, 6], F32, tag="bpm1")
            nc.vector.tensor_single_scalar(out=bpm1_sb[:], in_=bp_sb[:], scalar=1.0, op=ALU.subtract)
            bg_sb = bias_tile(b_g, 2, "bg")
            bc_sb = bias_tile(b_c, 1, "bc")

            def bias_row(par, n, tag):
                # [S, n] broadcast of a length-n bias (free-dim layout)
                brow = cp.tile([1, D], F32, tag="brow")
                row = brow[:, 0:n]
                nc.sync.dma_start(out=row[:], in_=par[:].unsqueeze(0))
                t = cp.tile([S, n], F32, tag=tag)
                nc.gpsimd.partition_broadcast(t[:], row[:])
                return t
            bhp_bc = bias_row(b_hp, D, "bhp")
            bhm_bc = bias_row(b_hm, D, "bhm")
            bhg_bc = bias_row(b_hg, D, "bhg")
            bc1_bc = bias_row(b_c1, D, "bc1")
            bc2_bc = bias_row(b_c2, 2, "bc2")

            ones_bf = cp.tile([128, 1], BF16, tag="ones_bf")
            nc.vector.memset(ones_bf[:], 1.0)

            # ---------------- setup: extended weights ----------------
            # W_ext[:, k, :] = [W rows | W@al | W@ar] (bf16)
            def build_ext(Wpar, alpar, arpar, kd, n_out, tag, sp):
                ext = cp.tile([128, kd, n_out + 2], BF16, tag=tag)
                al_bc = sp.tile([128, n_out], F32, tag="albc")
                ar_bc = sp.tile([128, n_out], F32, tag="arbc")
                nc.gpsimd.dma_start(out=al_bc[:], in_=alpar[:].partition_broadcast(128))
                nc.gpsimd.dma_start(out=ar_bc[:], in_=arpar[:].partition_broadcast(128))
                for k in range(kd):
                    nc.gpsimd.dma_start(out=ext[:, k, 0:n_out], in_=Wpar[k * 128:(k + 1) * 128, :])
                    tmp = sp.tile([128, n_out], F32, tag="wtmp")
                    red = sp.tile([128, 1], F32, tag="wred")
                    nc.vector.tensor_tensor(out=tmp[:], in0=ext[:, k, 0:n_out], in1=al_bc[:], op=ALU.mult)
                    nc.vector.reduce_sum(out=red[:], in_=tmp[:], axis=mybir.AxisListType.X)
                    nc.vector.tensor_copy(out=ext[:, k, n_out:n_out + 1], in_=red[:])
                    tmp2 = sp.tile([128, n_out], F32, tag="wtmp")
                    red2 = sp.tile([128, 1], F32, tag="wred")
                    nc.vector.tensor_tensor(out=tmp2[:], in0=ext[:, k, 0:n_out], in1=ar_bc[:], op=ALU.mult)
                    nc.vector.reduce_sum(out=red2[:], in_=tmp2[:], axis=mybir.AxisListType.X)
                    nc.vector.tensor_copy(out=ext[:, k, n_out + 1:n_out + 2], in_=red2[:])
                return ext

            with tc.tile_pool(name="setup", bufs=1) as sp:
                Wp_ext = build_ext(W_p, al_p, ar_p, KD, D, "wpext", sp)   # [128, 6, 770]
                Wg_ext = build_ext(W_g, al_g, ar_g, KD, FG, "wgext", sp)  # [128, 6, 258]

            Wc_sb = cp.tile([128, 2, DC], BF16, tag="wc")
            for k in range(2):
                nc.gpsimd.dma_start(out=Wc_sb[:, k, :], in_=W_c[k * 128:(k + 1) * 128, :])
            Whp_sb = cp.tile([128, D], F32, tag="whp")
            nc.sync.dma_start(out=Whp_sb[:], in_=W_hp[:])

            # ---------------- setup: DRAM staging (bf16) ----------------
            # (img0 + txt staged in the preamble below; img 1..7 prefetched
            # one sample ahead inside emit_A)
            txt_bf = dpool.tile([S * NT1, D], BF16)
            img_tiles = {}
            def stage_img(s):
                t = dstg.tile([NI, D], BF16, tag="imgbf")
                nc.gpsimd.dma_start(out=t[:], in_=img[s, 1:577, :])
                img_tiles[s] = t
            stage_img(0)
            nc.gpsimd.dma_start(out=txt_bf[:], in_=txt.rearrange("s t d -> (s t) d"))

            # text transposed: xtT [128, 6, 512]
            xtT = cp.tile([128, KD, S * NT1], BF16, tag="xtT")
            for k in range(KD):
                nc.sync.dma_start(out=xtT[:, k, :], in_=txt_bf[:, k * 128:(k + 1) * 128], transpose=True)

            # text h (all samples): h_t [128, 4, 768] bf16 + elr_t [128, 4, 2] f32
            h_t = cp.tile([128, 4, D], BF16, tag="h_t")
            elr_t = cp.tile([128, 4, 2], F32, tag="elr_t")
            for mt in range(4):
                ps = pmm.tile([128, 770], F32, tag="mm")
                for (c0, c1) in CG770:
                    for k in range(KD):
                        nc.tensor.matmul(ps[:, c0:c1], xtT[:, k, mt * 128:(mt + 1) * 128],
                                         Wp_ext[:, k, c0:c1], start=(k == 0), stop=(k == KD - 1))
                nc.vector.tensor_copy(out=h_t[:, mt, :], in_=ps[:, 0:D])
                nc.scalar.copy(out=elr_t[:, mt, :], in_=ps[:, D:D + 2])

            # head accumulators
            a0_all = hp.tile([128, S, 2], F32, tag="a0all")
            c_all = hp.tile([1, S], F32, tag="call")

            # ---------------- per-sample 3-stage pipeline ----------------
            stA1 = {}
            stA = {}
            stF = {}
            stB = {}

            def emit_A1(s):
                """Prefetch next image, transpose-load x^T, h_ext matmuls, er row."""
                if s + 1 < S:
                    stage_img(s + 1)
                my_img = img_tiles.pop(s)
                xiT = wp.tile([128, KD, NI], BF16, tag="xiT")
                for k in range(KD):
                    nc.sync.dma_start(out=xiT[:, k, :], in_=my_img[:, k * 128:(k + 1) * 128],
                                      transpose=True)
                h_sb = wp.tile([128, 5, D], BF16, tag="h_img")
                elr = wp.tile([128, 5, 2], F32, tag="elr_img")
                for n in range(5):
                    rows = IMG_ROWS[n]
                    ps = pmm.tile([128, 770], F32, tag="mm")
                    for (c0, c1) in CG770:
                        for k in range(KD):
                            nc.tensor.matmul(ps[:rows, c0:c1], xiT[:, k, n * 128:n * 128 + rows],
                                             Wp_ext[:, k, c0:c1], start=(k == 0), stop=(k == KD - 1))
                    nc.scalar.copy(out=h_sb[:rows, n, :], in_=ps[:rows, 0:D])
                    nc.scalar.copy(out=elr[:rows, n, :], in_=ps[:rows, D:D + 2])

                ps_er = pmm.tile([1, 576], F32, tag="mm")
                for n in range(5):
                    rows = IMG_ROWS[n]
                    nc.tensor.matmul(ps_er[0:1, n * 128:n * 128 + rows], elr[:rows, n, 1:2],
                                     ident_sb[:rows, 0:rows], start=True, stop=True)
                er_row = wp.tile([1, NI], F32, tag="er_row")
                nc.scalar.copy(out=er_row[:], in_=ps_er[0:1, 0:NI])
                er_bc = wp.tile([128, NI], F32, tag="er_bc")
                nc.gpsimd.partition_broadcast(er_bc[:], er_row[:])
                stA1[s] = (h_sb, elr, er_bc)

            def emit_A2(s):
                """q = exp(lrelu(el + er)) * mask  (ACT-heavy, emitted late)."""
                h_sb, elr, er_bc = stA1.pop(s)
                q_sb = wp.tile([128, 5, NI], BF16, tag="q_img")
                for i in range(5):
                    rows = IMG_ROWS[i]
                    t1 = wp.tile([128, NI], BF16, tag="t1")
                    nc.scalar.activation(out=t1[:rows], in_=er_bc[:rows], func=ACT.Prelu,
                                         bias=elr[:rows, i, 0:1], scale=1.0, alpha=0.2)
                    nc.scalar.activation(out=t1[:rows], in_=t1[:rows], func=ACT.Exp)
                    nc.gpsimd.tensor_tensor(out=q_sb[:rows, i, :], in0=t1[:rows],
                                            in1=mimg_sb[:rows, i, :], op=ALU.mult)
                stA[s] = (h_sb, elr, q_sb)

            def emit_B1a(s):
                """Image colsum -> rc broadcast -> q normalize (gpsimd)."""
                h_sb, elr, q_sb = stA[s]
                ps_cs = pmm.tile([1, 576], F32, tag="mm")
                for (c0, c1) in CG576:
                    for i in range(5):
                        rows = IMG_ROWS[i]
                        nc.tensor.matmul(ps_cs[0:1, c0:c1], ones_bf[:rows, :], q_sb[:rows, i, c0:c1],
                                         start=(i == 0), stop=(i == 4))
                cs_row = wp.tile([1, NI], F32, tag="cs_row")
                nc.scalar.copy(out=cs_row[:], in_=ps_cs[0:1, 0:NI])
                cs_bc = wp.tile([128, NI], F32, tag="cs_bc")
                nc.gpsimd.partition_broadcast(cs_bc[:], cs_row[:])
                rc_bc = cs_bc
                nc.vector.reciprocal_approx_fast(out=rc_bc[:], in_=cs_bc[:])
                for i in range(5):
                    rows = IMG_ROWS[i]
                    nc.vector.tensor_tensor(out=q_sb[:rows, i, :], in0=q_sb[:rows, i, :],
                                            in1=rc_bc[:rows], op=ALU.mult)

            def emit_B1b(s):
                """Image aggregation + bias + elu -> feat image columns; text attention."""
                h_sb, elr, q_sb = stA.pop(s)
                feat = wp.tile([128, KD, NF], BF16, tag="feat")
                for m in range(KD):
                    ps_a = pmm.tile([128, 576], F32, tag="mm")
                    for (c0, c1) in CG576:
                        for i in range(5):
                            rows = IMG_ROWS[i]
                            nc.tensor.matmul(ps_a[:, c0:c1], h_sb[:rows, i, m * 128:(m + 1) * 128],
                                             q_sb[:rows, i, c0:c1], start=(i == 0), stop=(i == 4))
                    mneg = wp.tile([128, NI], BF16, tag="mneg")
                    nc.vector.tensor_scalar(out=mneg[:], in0=ps_a[:, 0:NI], scalar1=bp_sb[:, m:m + 1],
                                            scalar2=0.0, op0=ALU.add, op1=ALU.min)
                    spos = wp.tile([128, NI], BF16, tag="spos")
                    nc.vector.tensor_scalar(out=spos[:], in0=ps_a[:, 0:NI], scalar1=bpm1_sb[:, m:m + 1],
                                            scalar2=-1.0, op0=ALU.add, op1=ALU.max)
                    nc.scalar.activation(out=mneg[:], in_=mneg[:], func=ACT.Exp)
                    nc.vector.tensor_tensor(out=feat[:, m, 0:NI], in0=mneg[:], in1=spos[:], op=ALU.add)

                # ---- text attention ----
                b0 = (s % 2) * 64
                tt = s // 2
                ps_ert = pmm.tile([1, 576], F32, tag="mm")
                nc.tensor.matmul(ps_ert[0:1, 0:64], elr_t[b0:b0 + 64, tt, 1:2],
                                 ident_sb[b0:b0 + 64, b0:b0 + 64], start=True, stop=True)
                ert_row = wp.tile([1, NT1], F32, tag="ert_row")
                nc.scalar.copy(out=ert_row[:], in_=ps_ert[0:1, 0:NT1])
                ert_bc = wp.tile([128, NT1], F32, tag="ert_bc")
                nc.gpsimd.partition_broadcast(ert_bc[:], ert_row[:])

                t1t = wp.tile([128, NT1], BF16, tag="t1t")
                nc.scalar.activation(out=t1t[b0:b0 + 64], in_=ert_bc[b0:b0 + 64], func=ACT.Prelu,
                                     bias=elr_t[b0:b0 + 64, tt, 0:1], scale=1.0, alpha=0.2)
                nc.scalar.activation(out=t1t[b0:b0 + 64], in_=t1t[b0:b0 + 64], func=ACT.Exp)
                q_t = wp.tile([128, NT1], BF16, tag="q_t")
                nc.vector.tensor_tensor(out=q_t[b0:b0 + 64], in0=t1t[b0:b0 + 64],
                                        in1=mtxt_sb[b0:b0 + 64], op=ALU.mult)

                ps_ct = pmm.tile([1, 576], F32, tag="mm")
                nc.tensor.matmul(ps_ct[0:1, 0:NT1], ones_bf[b0:b0 + 64, :], q_t[b0:b0 + 64],
                                 start=True, stop=True)
                ct_row = wp.tile([1, NT1], F32, tag="ct_row")
                nc.scalar.copy(out=ct_row[:], in_=ps_ct[0:1, 0:NT1])
                ct_bc = wp.tile([128, NT1], F32, tag="ct_bc")
                nc.gpsimd.partition_broadcast(ct_bc[:], ct_row[:])
                rct_bc = ct_bc
                nc.vector.reciprocal_approx_fast(out=rct_bc[:], in_=ct_bc[:])
                nc.vector.tensor_tensor(out=q_t[b0:b0 + 64], in0=q_t[b0:b0 + 64],
                                        in1=rct_bc[b0:b0 + 64], op=ALU.mult)

                for m in range(KD):
                    ps_at = pmm.tile([128, NT1], F32, tag="mm")
                    nc.tensor.matmul(ps_at[:, 0:NT1], h_t[b0:b0 + 64, tt, m * 128:(m + 1) * 128],
                                     q_t[b0:b0 + 64], start=True, stop=True)
                    mneg_t = wp.tile([128, NT1], BF16, tag="mneg_t")
                    nc.vector.tensor_scalar(out=mneg_t[:], in0=ps_at[:, 0:NT1], scalar1=bp_sb[:, m:m + 1],
                                            scalar2=0.0, op0=ALU.add, op1=ALU.min)
                    spos_t = wp.tile([128, NT1], BF16, tag="spos_t")
                    nc.vector.tensor_scalar(out=spos_t[:], in0=ps_at[:, 0:NT1], scalar1=bpm1_sb[:, m:m + 1],
                                            scalar2=-1.0, op0=ALU.add, op1=ALU.max)
                    nc.scalar.activation(out=mneg_t[:], in_=mneg_t[:], func=ACT.Exp)
                    nc.vector.tensor_tensor(out=feat[:, m, NI:NF], in0=mneg_t[:, 1:64],
                                            in1=spos_t[:, 1:64], op=ALU.add)
                stF[s] = feat

            def emit_B2(s):
                """Fused GAT h_g + q0 chain."""
                feat = stF.pop(s)
                h_g = wp.tile([128, 5, FG], BF16, tag="h_g")
                elg = wp.tile([128, 5, 2], F32, tag="elg")
                for n in range(5):
                    rows = FUS_ROWS[n]
                    ps_g = pmm.tile([128, FG + 2], F32, tag="mm")
                    for k in range(KD):
                        nc.tensor.matmul(ps_g[:rows, :], feat[:, k, n * 128:n * 128 + rows],
                                         Wg_ext[:, k, :], start=(k == 0), stop=(k == KD - 1))
                    nc.vector.tensor_copy(out=h_g[:rows, n, :], in_=ps_g[:rows, 0:FG])
                    nc.scalar.copy(out=elg[:rows, n, :], in_=ps_g[:rows, FG:FG + 2])

                er0_bc = wp.tile([128, 1], F32, tag="er0_bc")
                nc.gpsimd.partition_broadcast(er0_bc[:], elg[0:1, 0, 1:2])
                q0 = wp.tile([128, 5], F32, tag="q0")
                q0b = wp.tile([128, 5], BF16, tag="q0b")
                for n in range(5):
                    rows = FUS_ROWS[n]
                    nc.scalar.activation(out=q0[:rows, n:n + 1], in_=elg[:rows, n, 0:1],
                                         func=ACT.Prelu, bias=er0_bc[:rows], scale=1.0, alpha=0.2)
                    nc.scalar.activation(out=q0b[:rows, n:n + 1], in_=q0[:rows, n:n + 1], func=ACT.Exp)
                nc.vector.memset(q0b[0:1, 0:1], 0.0)
                stB[s] = (h_g, q0b)

            def emit_C(s):
                """Fused attention column 0: normalizer + aggregation."""
                h_g, q0b = stB.pop(s)
                ps_c = pmm.tile([1, 576], F32, tag="mm")
                for n in range(5):
                    rows = FUS_ROWS[n]
                    nc.tensor.matmul(ps_c[0:1, 0:1], q0b[:rows, n:n + 1], ones_bf[:rows, :],
                                     start=(n == 0), stop=(n == 4))
                ps_a0 = pmm.tile([128, 2], F32, tag="mm")
                for mfg in range(2):
                    for n in range(5):
                        rows = FUS_ROWS[n]
                        nc.tensor.matmul(ps_a0[:, mfg:mfg + 1],
                                         h_g[:rows, n, mfg * 128:(mfg + 1) * 128],
                                         q0b[:rows, n:n + 1], start=(n == 0), stop=(n == 4))
                nc.vector.tensor_copy(out=a0_all[:, s, :], in_=ps_a0[:, 0:2])
                nc.scalar.copy(out=c_all[0:1, s:s + 1], in_=ps_c[0:1, 0:1])

            emit_A1(0)
            emit_A2(0)
            emit_B1a(0)
            emit_A1(1)
            emit_B1b(0)
            emit_B2(0)
            emit_A2(1)
            for s in range(2, S):
                emit_B1a(s - 1)
                emit_A1(s)
                emit_B1b(s - 1)
                emit_B2(s - 1)
                emit_A2(s)
                emit_C(s - 2)
            emit_B1a(S - 1)
            emit_B1b(S - 1)
            emit_B2(S - 1)
            emit_C(S - 2)
            emit_C(S - 1)

            # ---------------- batched head (f32, sample-major [S, .] layout) ----
            rc0_row = hp.tile([1, S], F32, tag="rc0row")
            nc.vector.reciprocal_approx_fast(out=rc0_row[:], in_=c_all[:])
            rc0_bc = hp.tile([128, S], F32, tag="rc0bc")
            nc.gpsimd.partition_broadcast(rc0_bc[:], rc0_row[:])

            gat0 = hp.tile([128, 2, S], BF16, tag="gat0")
            for mfg in range(2):
                y0 = hp.tile([128, S], F32, tag="y0")
                nc.vector.tensor_tensor(out=y0[:], in0=a0_all[:, :, mfg], in1=rc0_bc[:], op=ALU.mult)
                m0 = hp.tile([128, S], F32, tag="m0")
                nc.vector.tensor_scalar(out=m0[:], in0=y0[:], scalar1=bg_sb[:, mfg:mfg + 1],
                                        scalar2=0.0, op0=ALU.add, op1=ALU.min)
                s0 = hp.tile([128, S], F32, tag="s0")
                nc.vector.tensor_scalar(out=s0[:], in0=y0[:], scalar1=bg_sb[:, mfg:mfg + 1],
                                        scalar2=0.0, op0=ALU.add, op1=ALU.max)
                e0 = hp.tile([128, S], F32, tag="e0")
                nc.scalar.activation(out=e0[:], in_=m0[:], func=ACT.Exp)
                nc.vector.scalar_tensor_tensor(out=gat0[:, mfg, :], in0=e0[:], scalar=-1.0,
                                               in1=s0[:], op0=ALU.add, op1=ALU.add)

            # ge^T [dc=128, S] = relu(W_c^T gat0 + b_c)
            ps_ge = pmm.tile([128, S], F32, tag="mm")
            for k in range(2):
                nc.tensor.matmul(ps_ge[:], Wc_sb[:, k, :], gat0[:, k, :], start=(k == 0), stop=(k == 1))
            ge_sb = hp.tile([128, S], F32, tag="ge")
            nc.scalar.activation(out=ge_sb[:], in_=ps_ge[:], func=ACT.Relu, bias=bc_sb[:, 0:1])

            # graph [S, 768] = ge^T.T @ W_hp + b_hp   (row-major, samples on partitions)
            ps_gr = pmm.tile([S, D], F32, tag="mm")
            for (c0, c1) in CG768:
                nc.tensor.matmul(ps_gr[0:S, c0:c1], ge_sb[:], Whp_sb[:, c0:c1], start=True, stop=True)
            graph_sb = hp.tile([S, D], F32, tag="graph")
            nc.vector.tensor_tensor(out=graph_sb[:], in0=ps_gr[0:S, :], in1=bhp_bc[:], op=ALU.add)

            # mul [S, 768] = relu(text CLS)
            cls_sb = hp.tile([S, D], F32, tag="cls")
            nc.sync.dma_start(out=cls_sb[:], in_=txt[:, 0, :])
            mul_sb = cls_sb
            nc.scalar.activation(out=mul_sb[:], in_=cls_sb[:], func=ACT.Relu)

            gum_sb = hp.tile([S, 2, D], F32, tag="gum")
            nc.sync.dma_start(out=gum_sb[:], in_=gum[:])

            # fusion^T tiles [128, S] x 12 via identity-matmul transposes
            fusT = hp.tile([128, 2 * KD, S], F32, tag="fusT")
            for m in range(KD):
                ps = pmm.tile([128, S], F32, tag="mm")
                nc.tensor.matmul(ps[:, 0:S], mul_sb[:, m * 128:(m + 1) * 128],
                                 ident_sb[0:S, 0:S], start=True, stop=True)
                nc.vector.tensor_copy(out=fusT[:, m, :], in_=ps[:, 0:S])
            for m in range(KD):
                ps = pmm.tile([128, S], F32, tag="mm")
                nc.tensor.matmul(ps[:, 0:S], graph_sb[:, m * 128:(m + 1) * 128],
                                 ident_sb[0:S, 0:S], start=True, stop=True)
                nc.vector.tensor_copy(out=fusT[:, KD + m, :], in_=ps[:, 0:S])

            # gates: sm/sg [S, 768] = sigmoid(fusion @ W + b); W streamed as rhs
            def gate(Wpar, bias_bc, tag):
                ps = pmm.tile([S, D], F32, tag="mm")
                for (c0, c1) in CG768:
                    for k in range(2 * KD):
                        wt = wsf.tile([128, 512], F32, tag="wsf")
                        nc.sync.dma_start(out=wt[:, 0:c1 - c0], in_=Wpar[k * 128:(k + 1) * 128, c0:c1])
                        nc.tensor.matmul(ps[0:S, c0:c1], fusT[:, k, :], wt[:, 0:c1 - c0],
                                         start=(k == 0), stop=(k == 2 * KD - 1))
                dst = hp.tile([S, D], F32, tag=tag)
                nc.vector.tensor_tensor(out=dst[:], in0=ps[0:S, :], in1=bias_bc[:], op=ALU.add)
                nc.scalar.activation(out=dst[:], in_=dst[:], func=ACT.Sigmoid)
                return dst
            sm_sb = gate(W_hm, bhm_bc, "sm")
            sg_sb = gate(W_hg, bhg_bc, "sg")

            # select: pre = (sm+g0 >= sg+g1) ? mul : graph
            nc.vector.tensor_tensor(out=sm_sb[:], in0=sm_sb[:], in1=gum_sb[:, 0, :], op=ALU.add)
            nc.vector.tensor_tensor(out=sg_sb[:], in0=sg_sb[:], in1=gum_sb[:, 1, :], op=ALU.add)
            dd = sm_sb
            nc.vector.tensor_tensor(out=dd[:], in0=sm_sb[:], in1=sg_sb[:], op=ALU.subtract)
            msk = hp.tile([S, D], U8, tag="msk")
            nc.vector.tensor_single_scalar(out=msk[:], in_=dd[:], scalar=0.0, op=ALU.is_ge)
            pre_sb = hp.tile([S, D], F32, tag="pre")
            nc.vector.select(out=pre_sb[:], mask=msk[:], on_true=mul_sb[:], on_false=graph_sb[:])

            # z = relu(pre @ W_c1 + b_c1):  pre^T tiles then W_c1 streamed as rhs
            preT = hp.tile([128, KD, S], BF16, tag="preT")
            for m in range(KD):
                ps = pmm.tile([128, S], F32, tag="mm")
                nc.tensor.matmul(ps[:, 0:S], pre_sb[:, m * 128:(m + 1) * 128],
                                 ident_sb[0:S, 0:S], start=True, stop=True)
                nc.vector.tensor_copy(out=preT[:, m, :], in_=ps[:, 0:S])
            ps_z = pmm.tile([S, D], F32, tag="mm")
            for (c0, c1) in CG768:
                for k in range(KD):
                    wt = wsp.tile([128, 512], BF16, tag="wst")
                    nc.gpsimd.dma_start(out=wt[:, 0:c1 - c0], in_=W_c1[k * 128:(k + 1) * 128, c0:c1])
                    nc.tensor.matmul(ps_z[0:S, c0:c1], preT[:, k, :], wt[:, 0:c1 - c0],
                                     start=(k == 0), stop=(k == KD - 1))
            z_sb = hp.tile([S, D], F32, tag="zb")
            nc.vector.tensor_tensor(out=z_sb[:], in0=ps_z[0:S, :], in1=bc1_bc[:], op=ALU.add)
            nc.scalar.activation(out=z_sb[:], in_=z_sb[:], func=ACT.Relu)

            # pred [S, 2] = z @ W_c2 + b_c2:  z^T tiles then W_c2 as rhs
            zT = hp.tile([128, KD, S], BF16, tag="zT")
            for m in range(KD):
                ps = pmm.tile([128, S], F32, tag="mm")
                nc.tensor.matmul(ps[:, 0:S], z_sb[:, m * 128:(m + 1) * 128],
                                 ident_sb[0:S, 0:S], start=True, stop=True)
                nc.vector.tensor_copy(out=zT[:, m, :], in_=ps[:, 0:S])
            w2 = hp.tile([128, KD, 2], BF16, tag="w2")
            nc.gpsimd.dma_start(out=w2[:], in_=W_c2.rearrange("(k p) c -> p k c", p=128))
            ps_p = pmm.tile([S, 2], F32, tag="mm")
            for k in range(KD):
                nc.tensor.matmul(ps_p[0:S, 0:2], zT[:, k, :], w2[:, k, :],
                                 start=(k == 0), stop=(k == KD - 1))
            pred_sb = hp.tile([S, 2], F32, tag="pred")
            nc.vector.tensor_tensor(out=pred_sb[:], in0=ps_p[0:S, 0:2], in1=bc2_bc[:], op=ALU.add)
            nc.sync.dma_start(out=out_p[:], in_=pred_sb[:])

    nc.compile()
    return nc


def _get_compiled():
    global _COMPILED
    if _COMPILED is None:
        _COMPILED = (_build(), _host_consts())
    return _COMPILED


def _make_in_maps(inputs, consts):
    arrs = {k: np.asarray(v, dtype=np.float32) for k, v in inputs.items()}
    in_maps = []
    for c in range(N_CORES):
        sl = slice(c * S, (c + 1) * S)
        m = {
            "image_embeds": arrs["image_embeds"][sl],
            "text_hidden": arrs["text_hidden"][sl],
            "gumbel": arrs["gumbel"][sl],
        }
        for k, v in arrs.items():
            if k not in m:
                m[k] = v
        m.update(consts)
        in_maps.append(m)
    return in_maps


def kernel(**inputs):
    nc, consts = _get_compiled()
    in_maps = _make_in_maps(inputs, consts)
    res = run_bass_kernel_spmd(nc, in_maps, list(range(N_CORES)))
    out = np.concatenate([res.results[c]["out"] for c in range(N_CORES)], axis=0)
    return out.astype(np.float32)


def run_traced(**inputs):
    """Like kernel(), but returns (output, exec_time_ns, trace_path)."""
    nc, consts = _get_compiled()
    in_maps = _make_in_maps(inputs, consts)
    res = run_bass_kernel_spmd(nc, in_maps, list(range(N_CORES)), trace=True)
    out = np.concatenate([res.results[c]["out"] for c in range(N_CORES)], axis=0)
    trace_path = res.instructions_and_trace[1] if res.instructions_and_trace else None
    return out.astype(np.float32), res.exec_time_ns, trace_path


# revision 18
# speedup vs baseline: 1.4052x; 1.4052x over previous
"""Trainium2 Bass kernel for the ALBEF-style GAT fusion model.

Shards batch B=64 across 8 NeuronCores (data parallel, 8 samples/core).
Key algorithmic facts exploited:
  - Only graph_embedding[:, 0, :] feeds the final head, so the fused GAT
    needs only attention column 0 (one softmax column), not the full
    639x639 attention.
  - GAT attention logits el/er are linear in the projection, so they are
    folded into the projection matmul as two extra weight columns.
  - The aggregation (alpha^T h) is emitted transposed (feat^T layout) so
    the next GAT's matmul needs no on-chip transpose.
Compute dtype: bf16 on the TensorEngine for the heavy matmuls, f32 for the
gumbel-gate / classifier head (argmax selection is precision-sensitive).
The per-sample work is software-pipelined in three stages so the
TensorEngine never stalls on the ScalarEngine softmax chain.
"""

import sys
import types

import numpy as np


def _install_ntff_hook():
    """Register the NTFF profiling hook that the trimmed antenv lacks.

    Without this, run_bass_kernel_spmd(trace=True) (or BASS_TRACE=1) dies on
    `import antenv.axon_hooks`. With it, tracing works via ctypes into
    libaxon_pjrt.so.
    """
    try:
        if 'antenv.axon_hooks' not in sys.modules:
            hooks = types.ModuleType('antenv.axon_hooks')
            _h = [None]
            hooks.set_axon_ntff_profile_hook = lambda h: _h.__setitem__(0, h)
            hooks.get_axon_ntff_profile_hook = lambda: _h[0]
            sys.modules['antenv.axon_hooks'] = hooks
            import antenv
            antenv.axon_hooks = hooks
        import trn_agent_boot.trn_boot as tb
        from antenv.axon_hooks import set_axon_ntff_profile_hook
        hook = tb._ntff_profile_via_ctypes('/opt/axon/libaxon_pjrt.so')
        if hook is not None:
            set_axon_ntff_profile_hook(hook)
    except Exception:
        pass


_install_ntff_hook()

import concourse.bacc as bacc
import concourse.tile as tile
from concourse import mybir
from concourse.bass_utils import run_bass_kernel_spmd

import ml_dtypes

F32 = mybir.dt.float32
BF16 = mybir.dt.bfloat16
U8 = mybir.dt.uint8
ALU = mybir.AluOpType
ACT = mybir.ActivationFunctionType

N_CORES = 8
B = 64
S = B // N_CORES          # samples per core
D = 768                   # hidden
FG = 256                  # GATConv out
DC = 128                  # classifier out
NI = 576                  # image nodes (CLS dropped)
NT1 = 64                  # text tokens incl CLS
NTN = 63                  # text nodes
NF = NI + NTN             # fused nodes = 639
KD = D // 128             # 6 k-tiles over hidden

IMG_ROWS = [128, 128, 128, 128, 64]          # image node tiles
FUS_ROWS = [128, 128, 128, 128, 127]         # fused node tiles
CG770 = [(0, 512), (512, 770)]
CG576 = [(0, 512), (512, 576)]
CG768 = [(0, 512), (512, 768)]

_COMPILED = None  # cached (nc, const_inputs)


def _host_consts():
    ident = np.eye(128, dtype=np.float32)
    # image diag mask, per i-tile: [5, 128, 576]
    mimg = np.ones((5, 128, NI), dtype=np.float32)
    for t in range(5):
        for p in range(128):
            i = t * 128 + p
            if i < NI:
                mimg[t, p, i] = 0.0
            else:
                mimg[t, p, :] = 0.0
    # text mask [128, 64]: token i=p%64 vs dst token j; zero CLS-as-src and diag
    mtxt = np.ones((128, NT1), dtype=np.float32)
    for p in range(128):
        i = p % 64
        mtxt[p, i] = 0.0
        if i == 0:
            mtxt[p, :] = 0.0
    return {
        "ident": ident,
        "mask_img": mimg.astype(ml_dtypes.bfloat16),
        "mask_txt": mtxt.astype(ml_dtypes.bfloat16),
    }


def _build():
    nc = bacc.Bacc()
    P = {}
    def dparam(name, shape, dtype=F32, out=False):
        P[name] = nc.declare_dram_parameter(name, list(shape), dtype, isOutput=out)
        return P[name]

    img = dparam("image_embeds", [S, 577, D])
    txt = dparam("text_hidden", [S, NT1, D])
    gum = dparam("gumbel", [S, 2, D])
    W_p = dparam("W_p", [D, D]); b_p = dparam("b_p", [D])
    al_p = dparam("al_p", [D]); ar_p = dparam("ar_p", [D])
    W_g = dparam("W_g", [D, FG]); b_g = dparam("b_g", [FG])
    al_g = dparam("al_g", [FG]); ar_g = dparam("ar_g", [FG])
    W_c = dparam("W_c", [FG, DC]); b_c = dparam("b_c", [DC])
    W_hp = dparam("W_hp", [DC, D]); b_hp = dparam("b_hp", [D])
    W_hm = dparam("W_hm", [2 * D, D]); b_hm = dparam("b_hm", [D])
    W_hg = dparam("W_hg", [2 * D, D]); b_hg = dparam("b_hg", [D])
    W_c1 = dparam("W_c1", [D, D]); b_c1 = dparam("b_c1", [D])
    W_c2 = dparam("W_c2", [D, 2]); b_c2 = dparam("b_c2", [2])
    ident = dparam("ident", [128, 128])
    mask_img = dparam("mask_img", [5, 128, NI], BF16)
    mask_txt = dparam("mask_txt", [128, NT1], BF16)
    out_p = dparam("out", [S, 2], out=True)

    with tile.TileContext(nc) as tc:
        with (
            tc.tile_pool(name="dram", bufs=1, space="DRAM") as dpool,
            tc.tile_pool(name="dstg", bufs=3, space="DRAM") as dstg,
            tc.tile_pool(name="consts", bufs=1) as cp,
            tc.tile_pool(name="work", bufs=2) as wp,
            tc.tile_pool(name="head", bufs=1) as hp,
            tc.tile_pool(name="wstream", bufs=6) as wsp,
            tc.tile_pool(name="wsf32", bufs=6) as wsf,
            tc.tile_pool(name="pmm", bufs=4, space="PSUM") as pmm,
        ):
            # ---------------- setup: constants ----------------
            ident_sb = cp.tile([128, 128], F32, tag="ident")
            nc.sync.dma_start(out=ident_sb[:], in_=ident[:])
            mimg_sb = cp.tile([128, 5, NI], BF16, tag="mimg")
            nc.sync.dma_start(out=mimg_sb[:], in_=mask_img.rearrange("t p j -> p t j"))
            mtxt_sb = cp.tile([128, NT1], BF16, tag="mtxt")
            nc.sync.dma_start(out=mtxt_sb[:], in_=mask_txt[:])

            onesf1 = cp.tile([1, 1], F32, tag="onesf1")
            nc.vector.memset(onesf1[:], 1.0)

            def bias_tile(par, n, tag, parts=128):
                # contiguous row load + K=1 matmul transpose (avoids 4B-strided DMA)
                brow2 = cp.tile([1, D], F32, tag="brow")
                nc.sync.dma_start(out=brow2[0:1, 0:n * parts], in_=par[:].unsqueeze(0))
                ps_b = pmm.tile([128, 6], F32, tag="mm")
                for m in range(n):
                    nc.tensor.matmul(ps_b[0:parts, m:m + 1], brow2[0:1, m * parts:(m + 1) * parts],
                                     onesf1[:], start=True, stop=True)
                t = cp.tile([parts, n], F32, tag=tag)
                nc.scalar.copy(out=t[:], in_=ps_b[0:parts, 0:n])
                return t
            bp_sb = bias_tile(b_p, 6, "bp")
            bpm1_sb = cp.tile([128, 6], F32, tag="bpm1")
            nc.vector.tensor_single_scalar(out=bpm1_sb[:], in_=bp_sb[:], scalar=1.0, op=ALU.subtract)
            bg_sb = bias_tile(b_g, 2, "bg")
            bc_sb = bias_tile(b_c, 1, "bc")

            def bias_row(par, n, tag):
                # [S, n] broadcast of a length-n bias (free-dim layout)
                brow = cp.tile([1, D], F32, tag="brow")
                row = brow[:, 0:n]
                nc.sync.dma_start(out=row[:], in_=par[:].unsqueeze(0))
                t = cp.tile([S, n], F32, tag=tag)
                nc.gpsimd.partition_broadcast(t[:], row[:])
                return t
            bhp_bc = bias_row(b_hp, D, "bhp")
            bhm_bc = bias_row(b_hm, D, "bhm")
            bhg_bc = bias_row(b_hg, D, "bhg")
            bc1_bc = bias_row(b_c1, D, "bc1")
            bc2_bc = bias_row(b_c2, 2, "bc2")

            ones_bf = cp.tile([128, 1], BF16, tag="ones_bf")
            nc.vector.memset(ones_bf[:], 1.0)

            # ---------------- setup: extended weights ----------------
            # W_ext[:, k, :] = [W rows | W@al | W@ar] (bf16)
            def build_ext(Wpar, alpar, arpar, kd, n_out, tag, sp):
                ext = cp.tile([128, kd, n_out + 2], BF16, tag=tag)
                al_bc = sp.tile([128, n_out], F32, tag="albc")
                ar_bc = sp.tile([128, n_out], F32, tag="arbc")
                nc.gpsimd.dma_start(out=al_bc[:], in_=alpar[:].partition_broadcast(128))
                nc.gpsimd.dma_start(out=ar_bc[:], in_=arpar[:].partition_broadcast(128))
                for k in range(kd):
                    nc.gpsimd.dma_start(out=ext[:, k, 0:n_out], in_=Wpar[k * 128:(k + 1) * 128, :])
                    tmp = sp.tile([128, n_out], F32, tag="wtmp")
                    red = sp.tile([128, 1], F32, tag="wred")
                    nc.vector.tensor_tensor(out=tmp[:], in0=ext[:, k, 0:n_out], in1=al_bc[:], op=ALU.mult)
                    nc.vector.reduce_sum(out=red[:], in_=tmp[:], axis=mybir.AxisListType.X)
                    nc.vector.tensor_copy(out=ext[:, k, n_out:n_out + 1], in_=red[:])
                    tmp2 = sp.tile([128, n_out], F32, tag="wtmp")
                    red2 = sp.tile([128, 1], F32, tag="wred")
                    nc.vector.tensor_tensor(out=tmp2[:], in0=ext[:, k, 0:n_out], in1=ar_bc[:], op=ALU.mult)
                    nc.vector.reduce_sum(out=red2[:], in_=tmp2[:], axis=mybir.AxisListType.X)
                    nc.vector.tensor_copy(out=ext[:, k, n_out + 1:n_out + 2], in_=red2[:])
                return ext

            with tc.tile_pool(name="setup", bufs=1) as sp:
                Wp_ext = build_ext(W_p, al_p, ar_p, KD, D, "wpext", sp)   # [128, 6, 770]
                Wg_ext = build_ext(W_g, al_g, ar_g, KD, FG, "wgext", sp)  # [128, 6, 258]

            Wc_sb = cp.tile([128, 2, DC], BF16, tag="wc")
            for k in range(2):
                nc.gpsimd.dma_start(out=Wc_sb[:, k, :], in_=W_c[k * 128:(k + 1) * 128, :])
            Whp_sb = cp.tile([128, D], F32, tag="whp")
            nc.sync.dma_start(out=Whp_sb[:], in_=W_hp[:])

            # ---------------- setup: DRAM staging (bf16) ----------------
            # (img0 + txt staged in the preamble below; img 1..7 prefetched
            # one sample ahead inside emit_A)
            txt_bf = dpool.tile([S * NT1, D], BF16)
            img_tiles = {}
            def stage_img(s):
                t = dstg.tile([NI, D], BF16, tag="imgbf")
                nc.gpsimd.dma_start(out=t[:], in_=img[s, 1:577, :])
                img_tiles[s] = t
            stage_img(0)
            nc.gpsimd.dma_start(out=txt_bf[:], in_=txt.rearrange("s t d -> (s t) d"))

            # text transposed: xtT [128, 6, 512]
            xtT = cp.tile([128, KD, S * NT1], BF16, tag="xtT")
            for k in range(KD):
                nc.sync.dma_start(out=xtT[:, k, :], in_=txt_bf[:, k * 128:(k + 1) * 128], transpose=True)

            # text h (all samples): h_t [128, 4, 768] bf16 + elr_t [128, 4, 2] f32
            h_t = cp.tile([128, 4, D], BF16, tag="h_t")
            elr_t = cp.tile([128, 4, 2], F32, tag="elr_t")
            for mt in range(4):
                ps = pmm.tile([128, 770], F32, tag="mm")
                for (c0, c1) in CG770:
                    for k in range(KD):
                        nc.tensor.matmul(ps[:, c0:c1], xtT[:, k, mt * 128:(mt + 1) * 128],
                                         Wp_ext[:, k, c0:c1], start=(k == 0), stop=(k == KD - 1))
                nc.vector.tensor_copy(out=h_t[:, mt, :], in_=ps[:, 0:D])
                nc.scalar.copy(out=elr_t[:, mt, :], in_=ps[:, D:D + 2])

            # head accumulators
            a0_all = hp.tile([128, S, 2], F32, tag="a0all")
            c_all = hp.tile([1, S], F32, tag="call")

            # ---------------- per-sample 3-stage pipeline ----------------
            stA1 = {}
            stA = {}
            stF = {}
            stB = {}

            def emit_A1(s):
                """Prefetch next image, transpose-load x^T, h_ext matmuls, er row."""
                if s + 1 < S:
                    stage_img(s + 1)
                my_img = img_tiles.pop(s)
                xiT = wp.tile([128, KD, NI], BF16, tag="xiT")
                for k in range(KD):
                    nc.sync.dma_start(out=xiT[:, k, :], in_=my_img[:, k * 128:(k + 1) * 128],
                                      transpose=True)
                h_sb = wp.tile([128, 5, D], BF16, tag="h_img")
                elr = wp.tile([128, 5, 2], F32, tag="elr_img")
                for n in range(5):
                    rows = IMG_ROWS[n]
                    ps = pmm.tile([128, 770], F32, tag="mm")
                    for (c0, c1) in CG770:
                        for k in range(KD):
                            nc.tensor.matmul(ps[:rows, c0:c1], xiT[:, k, n * 128:n * 128 + rows],
                                             Wp_ext[:, k, c0:c1], start=(k == 0), stop=(k == KD - 1))
                    nc.vector.tensor_copy(out=h_sb[:rows, n, :], in_=ps[:rows, 0:D])
                    nc.scalar.copy(out=elr[:rows, n, :], in_=ps[:rows, D:D + 2])

                ps_er = pmm.tile([1, 576], F32, tag="mm")
                for n in range(5):
                    rows = IMG_ROWS[n]
                    nc.tensor.matmul(ps_er[0:1, n * 128:n * 128 + rows], elr[:rows, n, 1:2],
                                     ident_sb[:rows, 0:rows], start=True, stop=True)
                er_row = wp.tile([1, NI], F32, tag="er_row")
                nc.scalar.copy(out=er_row[:], in_=ps_er[0:1, 0:NI])
                er_bc = wp.tile([128, NI], F32, tag="er_bc")
                nc.gpsimd.partition_broadcast(er_bc[:], er_row[:])
                stA1[s] = (h_sb, elr, er_bc)

            def emit_A2(s):
                """q = exp(lrelu(el + er)) * mask  (ACT-heavy, emitted late)."""
                h_sb, elr, er_bc = stA1.pop(s)
                q_sb = wp.tile([128, 5, NI], BF16, tag="q_img")
                for i in range(5):
                    rows = IMG_ROWS[i]
                    t1 = wp.tile([128, NI], BF16, tag="t1")
                    nc.scalar.activation(out=t1[:rows], in_=er_bc[:rows], func=ACT.Prelu,
                                         bias=elr[:rows, i, 0:1], scale=1.0, alpha=0.2)
                    nc.scalar.activation(out=t1[:rows], in_=t1[:rows], func=ACT.Exp)
                    nc.vector.tensor_tensor(out=q_sb[:rows, i, :], in0=t1[:rows],
                                            in1=mimg_sb[:rows, i, :], op=ALU.mult)
                stA[s] = (h_sb, elr, q_sb)

            def emit_B1a(s):
                """Image colsum -> rc broadcast -> q normalize (gpsimd)."""
                h_sb, elr, q_sb = stA[s]
                ps_cs = pmm.tile([1, 576], F32, tag="mm")
                for (c0, c1) in CG576:
                    for i in range(5):
                        rows = IMG_ROWS[i]
                        nc.tensor.matmul(ps_cs[0:1, c0:c1], ones_bf[:rows, :], q_sb[:rows, i, c0:c1],
                                         start=(i == 0), stop=(i == 4))
                cs_row = wp.tile([1, NI], F32, tag="cs_row")
                nc.scalar.copy(out=cs_row[:], in_=ps_cs[0:1, 0:NI])
                cs_bc = wp.tile([128, NI], F32, tag="cs_bc")
                nc.gpsimd.partition_broadcast(cs_bc[:], cs_row[:])
                rc_bc = cs_bc
                nc.vector.reciprocal_approx_fast(out=rc_bc[:], in_=cs_bc[:])
                for i in range(5):
                    rows = IMG_ROWS[i]
                    nc.vector.tensor_tensor(out=q_sb[:rows, i, :], in0=q_sb[:rows, i, :],
                                            in1=rc_bc[:rows], op=ALU.mult)

            def emit_B1b(s):
                """Image aggregation + bias + elu -> feat image columns; text attention."""
                h_sb, elr, q_sb = stA.pop(s)
                feat = wp.tile([128, KD, NF], BF16, tag="feat")
                for m in range(KD):
                    ps_a = pmm.tile([128, 576], F32, tag="mm")
                    for (c0, c1) in CG576:
                        for i in range(5):
                            rows = IMG_ROWS[i]
                            nc.tensor.matmul(ps_a[:, c0:c1], h_sb[:rows, i, m * 128:(m + 1) * 128],
                                             q_sb[:rows, i, c0:c1], start=(i == 0), stop=(i == 4))
                    mneg = wp.tile([128, NI], BF16, tag="mneg")
                    nc.vector.tensor_scalar(out=mneg[:], in0=ps_a[:, 0:NI], scalar1=bp_sb[:, m:m + 1],
                                            scalar2=0.0, op0=ALU.add, op1=ALU.min)
                    spos = wp.tile([128, NI], BF16, tag="spos")
                    nc.vector.tensor_scalar(out=spos[:], in0=ps_a[:, 0:NI], scalar1=bpm1_sb[:, m:m + 1],
                                            scalar2=-1.0, op0=ALU.add, op1=ALU.max)
                    nc.scalar.activation(out=mneg[:], in_=mneg[:], func=ACT.Exp)
                    nc.vector.tensor_tensor(out=feat[:, m, 0:NI], in0=mneg[:], in1=spos[:], op=ALU.add)

                # ---- text attention ----
                b0 = (s % 2) * 64
                tt = s // 2
                ps_ert = pmm.tile([1, 576], F32, tag="mm")
                nc.tensor.matmul(ps_ert[0:1, 0:64], elr_t[b0:b0 + 64, tt, 1:2],
                                 ident_sb[b0:b0 + 64, b0:b0 + 64], start=True, stop=True)
                ert_row = wp.tile([1, NT1], F32, tag="ert_row")
                nc.scalar.copy(out=ert_row[:], in_=ps_ert[0:1, 0:NT1])
                ert_bc = wp.tile([128, NT1], F32, tag="ert_bc")
                nc.gpsimd.partition_broadcast(ert_bc[:], ert_row[:])

                t1t = wp.tile([128, NT1], BF16, tag="t1t")
                nc.scalar.activation(out=t1t[b0:b0 + 64], in_=ert_bc[b0:b0 + 64], func=ACT.Prelu,
                                     bias=elr_t[b0:b0 + 64, tt, 0:1], scale=1.0, alpha=0.2)
                nc.scalar.activation(out=t1t[b0:b0 + 64], in_=t1t[b0:b0 + 64], func=ACT.Exp)
                q_t = wp.tile([128, NT1], BF16, tag="q_t")
                nc.vector.tensor_tensor(out=q_t[b0:b0 + 64], in0=t1t[b0:b0 + 64],
                                        in1=mtxt_sb[b0:b0 + 64], op=ALU.mult)

                ps_ct = pmm.tile([1, 576], F32, tag="mm")
                nc.tensor.matmul(ps_ct[0:1, 0:NT1], ones_bf[b0:b0 + 64, :], q_t[b0:b0 + 64],
                                 start=True, stop=True)
                ct_row = wp.tile([1, NT1], F32, tag="ct_row")
                nc.scalar.copy(out=ct_row[:], in_=ps_ct[0:1, 0:NT1])
                ct_bc = wp.tile([128, NT1], F32, tag="ct_bc")
                nc.gpsimd.partition_broadcast(ct_bc[:], ct_row[:])
                rct_bc = ct_bc
                nc.vector.reciprocal_approx_fast(out=rct_bc[:], in_=ct_bc[:])
                nc.vector.tensor_tensor(out=q_t[b0:b0 + 64], in0=q_t[b0:b0 + 64],
                                        in1=rct_bc[b0:b0 + 64], op=ALU.mult)

                for m in range(KD):
                    ps_at = pmm.tile([128, NT1], F32, tag="mm")
                    nc.tensor.matmul(ps_at[:, 0:NT1], h_t[b0:b0 + 64, tt, m * 128:(m + 1) * 128],
                                     q_t[b0:b0 + 64], start=True, stop=True)
                    mneg_t = wp.tile([128, NT1], BF16, tag="mneg_t")
                    nc.vector.tensor_scalar(out=mneg_t[:], in0=ps_at[:, 0:NT1], scalar1=bp_sb[:, m:m + 1],
                                            scalar2=0.0, op0=ALU.add, op1=ALU.min)
                    spos_t = wp.tile([128, NT1], BF16, tag="spos_t")
                    nc.vector.tensor_scalar(out=spos_t[:], in0=ps_at[:, 0:NT1], scalar1=bpm1_sb[:, m:m + 1],
                                            scalar2=-1.0, op0=ALU.add, op1=ALU.max)
                    nc.scalar.activation(out=mneg_t[:], in_=mneg_t[:], func=ACT.Exp)
                    nc.vector.tensor_tensor(out=feat[:, m, NI:NF], in0=mneg_t[:, 1:64],
                                            in1=spos_t[:, 1:64], op=ALU.add)
                stF[s] = feat

            def emit_B2(s):
                """Fused GAT h_g + q0 chain."""
                feat = stF.pop(s)
                h_g = wp.tile([128, 5, FG], BF16, tag="h_g")
                elg = wp.tile([128, 5, 2], F32, tag="elg")
                for n in range(5):
                    rows = FUS_ROWS[n]
                    ps_g = pmm.tile([128, FG + 2], F32, tag="mm")
                    for k in range(KD):
                        nc.tensor.matmul(ps_g[:rows, :], feat[:, k, n * 128:n * 128 + rows],
                                         Wg_ext[:, k, :], start=(k == 0), stop=(k == KD - 1))
                    nc.vector.tensor_copy(out=h_g[:rows, n, :], in_=ps_g[:rows, 0:FG])
                    nc.scalar.copy(out=elg[:rows, n, :], in_=ps_g[:rows, FG:FG + 2])

                er0_bc = wp.tile([128, 1], F32, tag="er0_bc")
                nc.gpsimd.partition_broadcast(er0_bc[:], elg[0:1, 0, 1:2])
                q0 = wp.tile([128, 5], F32, tag="q0")
                q0b = wp.tile([128, 5], BF16, tag="q0b")
                for n in range(5):
                    rows = FUS_ROWS[n]
                    nc.scalar.activation(out=q0[:rows, n:n + 1], in_=elg[:rows, n, 0:1],
                                         func=ACT.Prelu, bias=er0_bc[:rows], scale=1.0, alpha=0.2)
                    nc.scalar.activation(out=q0b[:rows, n:n + 1], in_=q0[:rows, n:n + 1], func=ACT.Exp)
                nc.vector.memset(q0b[0:1, 0:1], 0.0)
                stB[s] = (h_g, q0b)

            def emit_C(s):
                """Fused attention column 0: normalizer + aggregation."""
                h_g, q0b = stB.pop(s)
                ps_c = pmm.tile([1, 576], F32, tag="mm")
                for n in range(5):
                    rows = FUS_ROWS[n]
                    nc.tensor.matmul(ps_c[0:1, 0:1], q0b[:rows, n:n + 1], ones_bf[:rows, :],
                                     start=(n == 0), stop=(n == 4))
                ps_a0 = pmm.tile([128, 2], F32, tag="mm")
                for mfg in range(2):
                    for n in range(5):
                        rows = FUS_ROWS[n]
                        nc.tensor.matmul(ps_a0[:, mfg:mfg + 1],
                                         h_g[:rows, n, mfg * 128:(mfg + 1) * 128],
                                         q0b[:rows, n:n + 1], start=(n == 0), stop=(n == 4))
                nc.vector.tensor_copy(out=a0_all[:, s, :], in_=ps_a0[:, 0:2])
                nc.scalar.copy(out=c_all[0:1, s:s + 1], in_=ps_c[0:1, 0:1])

            emit_A1(0)
            emit_A2(0)
            emit_B1a(0)
            emit_A1(1)
            emit_B1b(0)
            emit_B2(0)
            emit_A2(1)
            for s in range(2, S):
                emit_B1a(s - 1)
                emit_A1(s)
                emit_B1b(s - 1)
                emit_B2(s - 1)
                emit_A2(s)
                emit_C(s - 2)
            emit_B1a(S - 1)
            emit_B1b(S - 1)
            emit_B2(S - 1)
            emit_C(S - 2)
            emit_C(S - 1)

            # ---------------- batched head (f32, sample-major [S, .] layout) ----
            rc0_row = hp.tile([1, S], F32, tag="rc0row")
            nc.vector.reciprocal_approx_fast(out=rc0_row[:], in_=c_all[:])
            rc0_bc = hp.tile([128, S], F32, tag="rc0bc")
            nc.gpsimd.partition_broadcast(rc0_bc[:], rc0_row[:])

            gat0 = hp.tile([128, 2, S], BF16, tag="gat0")
            for mfg in range(2):
                y0 = hp.tile([128, S], F32, tag="y0")
                nc.vector.tensor_tensor(out=y0[:], in0=a0_all[:, :, mfg], in1=rc0_bc[:], op=ALU.mult)
                m0 = hp.tile([128, S], F32, tag="m0")
                nc.vector.tensor_scalar(out=m0[:], in0=y0[:], scalar1=bg_sb[:, mfg:mfg + 1],
                                        scalar2=0.0, op0=ALU.add, op1=ALU.min)
                s0 = hp.tile([128, S], F32, tag="s0")
                nc.vector.tensor_scalar(out=s0[:], in0=y0[:], scalar1=bg_sb[:, mfg:mfg + 1],
                                        scalar2=0.0, op0=ALU.add, op1=ALU.max)
                e0 = hp.tile([128, S], F32, tag="e0")
                nc.scalar.activation(out=e0[:], in_=m0[:], func=ACT.Exp)
                nc.vector.scalar_tensor_tensor(out=gat0[:, mfg, :], in0=e0[:], scalar=-1.0,
                                               in1=s0[:], op0=ALU.add, op1=ALU.add)

            # ge^T [dc=128, S] = relu(W_c^T gat0 + b_c)
            ps_ge = pmm.tile([128, S], F32, tag="mm")
            for k in range(2):
                nc.tensor.matmul(ps_ge[:], Wc_sb[:, k, :], gat0[:, k, :], start=(k == 0), stop=(k == 1))
            ge_sb = hp.tile([128, S], F32, tag="ge")
            nc.scalar.activation(out=ge_sb[:], in_=ps_ge[:], func=ACT.Relu, bias=bc_sb[:, 0:1])

            # graph [S, 768] = ge^T.T @ W_hp + b_hp   (row-major, samples on partitions)
            ps_gr = pmm.tile([S, D], F32, tag="mm")
            for (c0, c1) in CG768:
                nc.tensor.matmul(ps_gr[0:S, c0:c1], ge_sb[:], Whp_sb[:, c0:c1], start=True, stop=True)
            graph_sb = hp.tile([S, D], F32, tag="graph")
            nc.vector.tensor_tensor(out=graph_sb[:], in0=ps_gr[0:S, :], in1=bhp_bc[:], op=ALU.add)

            # mul [S, 768] = relu(text CLS)
            cls_sb = hp.tile([S, D], F32, tag="cls")
            nc.sync.dma_start(out=cls_sb[:], in_=txt[:, 0, :])
            mul_sb = cls_sb
            nc.scalar.activation(out=mul_sb[:], in_=cls_sb[:], func=ACT.Relu)

            gum_sb = hp.tile([S, 2, D], F32, tag="gum")
            nc.sync.dma_start(out=gum_sb[:], in_=gum[:])

            # fusion^T tiles [128, S] x 12 via identity-matmul transposes
            fusT = hp.tile([128, 2 * KD, S], F32, tag="fusT")
            for m in range(KD):
                ps = pmm.tile([128, S], F32, tag="mm")
                nc.tensor.matmul(ps[:, 0:S], mul_sb[:, m * 128:(m + 1) * 128],
                                 ident_sb[0:S, 0:S], start=True, stop=True)
                nc.vector.tensor_copy(out=fusT[:, m, :], in_=ps[:, 0:S])
            for m in range(KD):
                ps = pmm.tile([128, S], F32, tag="mm")
                nc.tensor.matmul(ps[:, 0:S], graph_sb[:, m * 128:(m + 1) * 128],
                                 ident_sb[0:S, 0:S], start=True, stop=True)
                nc.vector.tensor_copy(out=fusT[:, KD + m, :], in_=ps[:, 0:S])

            # gates: sm/sg [S, 768] = sigmoid(fusion @ W + b); W streamed as rhs
            def gate(Wpar, bias_bc, tag):
                ps = pmm.tile([S, D], F32, tag="mm")
                for (c0, c1) in CG768:
                    for k in range(2 * KD):
                        wt = wsf.tile([128, 512], F32, tag="wsf")
                        nc.sync.dma_start(out=wt[:, 0:c1 - c0], in_=Wpar[k * 128:(k + 1) * 128, c0:c1])
                        nc.tensor.matmul(ps[0:S, c0:c1], fusT[:, k, :], wt[:, 0:c1 - c0],
                                         start=(k == 0), stop=(k == 2 * KD - 1))
                dst = hp.tile([S, D], F32, tag=tag)
                nc.vector.tensor_tensor(out=dst[:], in0=ps[0:S, :], in1=bias_bc[:], op=ALU.add)
                nc.scalar.activation(out=dst[:], in_=dst[:], func=ACT.Sigmoid)
                return dst
            sm_sb = gate(W_hm, bhm_bc, "sm")
            sg_sb = gate(W_hg, bhg_bc, "sg")

            # select: pre = (sm+g0 >= sg+g1) ? mul : graph
            nc.vector.tensor_tensor(out=sm_sb[:], in0=sm_sb[:], in1=gum_sb[:, 0, :], op=ALU.add)
            nc.vector.tensor_tensor(out=sg_sb[:], in0=sg_sb[:], in1=gum_sb[:, 1, :], op=ALU.add)
            dd = sm_sb
            nc.vector.tensor_tensor(out=dd[:], in0=sm_sb[:], in1=sg_sb[:], op=ALU.subtract)
            msk = hp.tile([S, D], U8, tag="msk")
            nc.vector.tensor_single_scalar(out=msk[:], in_=dd[:], scalar=0.0, op=ALU.is_ge)
            pre_sb = hp.tile([S, D], F32, tag="pre")
            nc.vector.select(out=pre_sb[:], mask=msk[:], on_true=mul_sb[:], on_false=graph_sb[:])

            # z = relu(pre @ W_c1 + b_c1):  pre^T tiles then W_c1 streamed as rhs
            preT = hp.tile([128, KD, S], BF16, tag="preT")
            for m in range(KD):
                ps = pmm.tile([128, S], F32, tag="mm")
                nc.tensor.matmul(ps[:, 0:S], pre_sb[:, m * 128:(m + 1) * 128],
                                 ident_sb[0:S, 0:S], start=True, stop=True)
                nc.vector.tensor_copy(out=preT[:, m, :], in_=ps[:, 0:S])
            ps_z = pmm.tile([S, D], F32, tag="mm")
            for (c0, c1) in CG768:
                for k in range(KD):
                    wt = wsp.tile([128, 512], BF16, tag="wst")
                    nc.gpsimd.dma_start(out=wt[:, 0:c1 - c0], in_=W_c1[k * 128:(k + 1) * 128, c0:c1])
                    nc.tensor.matmul(ps_z[0:S, c0:c1], preT[:, k, :], wt[:, 0:c1 - c0],
                                     start=(k == 0), stop=(k == KD - 1))
            z_sb = hp.tile([S, D], F32, tag="zb")
            nc.vector.tensor_tensor(out=z_sb[:], in0=ps_z[0:S, :], in1=bc1_bc[:], op=ALU.add)
            nc.scalar.activation(out=z_sb[:], in_=z_sb[:], func=ACT.Relu)

            # pred [S, 2] = z @ W_c2 + b_c2:  z^T tiles then W_c2 as rhs
            zT = hp.tile([128, KD, S], BF16, tag="zT")
            for m in range(KD):
                ps = pmm.tile([128, S], F32, tag="mm")
                nc.tensor.matmul(ps[:, 0:S], z_sb[:, m * 128:(m + 1) * 128],
                                 ident_sb[0:S, 0:S], start=True, stop=True)
                nc.vector.tensor_copy(out=zT[:, m, :], in_=ps[:, 0:S])
            w2 = hp.tile([128, KD, 2], BF16, tag="w2")
            nc.gpsimd.dma_start(out=w2[:], in_=W_c2.rearrange("(k p) c -> p k c", p=128))
            ps_p = pmm.tile([S, 2], F32, tag="mm")
            for k in range(KD):
                nc.tensor.matmul(ps_p[0:S, 0:2], zT[:, k, :], w2[:, k, :],
                                 start=(k == 0), stop=(k == KD - 1))
            pred_sb = hp.tile([S, 2], F32, tag="pred")
            nc.vector.tensor_tensor(out=pred_sb[:], in0=ps_p[0:S, 0:2], in1=bc2_bc[:], op=ALU.add)
            nc.sync.dma_start(out=out_p[:], in_=pred_sb[:])

    nc.compile()
    return nc


def _get_compiled():
    global _COMPILED
    if _COMPILED is None:
        _COMPILED = (_build(), _host_consts())
    return _COMPILED


def _make_in_maps(inputs, consts):
    arrs = {k: np.asarray(v, dtype=np.float32) for k, v in inputs.items()}
    in_maps = []
    for c in range(N_CORES):
        sl = slice(c * S, (c + 1) * S)
        m = {
            "image_embeds": arrs["image_embeds"][sl],
            "text_hidden": arrs["text_hidden"][sl],
            "gumbel": arrs["gumbel"][sl],
        }
        for k, v in arrs.items():
            if k not in m:
                m[k] = v
        m.update(consts)
        in_maps.append(m)
    return in_maps


def kernel(**inputs):
    nc, consts = _get_compiled()
    in_maps = _make_in_maps(inputs, consts)
    res = run_bass_kernel_spmd(nc, in_maps, list(range(N_CORES)))
    out = np.concatenate([res.results[c]["out"] for c in range(N_CORES)], axis=0)
    return out.astype(np.float32)


def run_traced(**inputs):
    """Like kernel(), but returns (output, exec_time_ns, trace_path)."""
    nc, consts = _get_compiled()
    in_maps = _make_in_maps(inputs, consts)
    res = run_bass_kernel_spmd(nc, in_maps, list(range(N_CORES)), trace=True)
    out = np.concatenate([res.results[c]["out"] for c in range(N_CORES)], axis=0)
    trace_path = res.instructions_and_trace[1] if res.instructions_and_trace else None
    return out.astype(np.float32), res.exec_time_ns, trace_path
